# revision 72
# baseline (speedup 1.0000x reference)
"""2-layer GAT (graph attention) forward, distributed across 8 Trainium2 NeuronCores.

Sharding: edges are partitioned by destination-node range (host-side sort by dst);
each core owns N/8 destination nodes and all edges pointing into them, so the
segment softmax and the scatter-add reduction are core-local.  Node features are
computed data-parallel over the owning core's nodes and replicated via AllGather.
Per-edge message aggregation is done as gather (indirect DMA of h[src] rows) +
one-hot matmul scatter into PSUM.  Only the tiny per-graph pooling sums need an
AllReduce at the end.

Key implementation choices (v2):
 - gather rows are 1280 B bf16: [h (512, c-major) | a_s (8) | a_d (8) | pad].
   h stays bf16 (fp8 rows cut traffic 40% but drop the DVE weight-multiply
   from 2x to 1x mode and 8-bit gathers halve the SWDGE descriptor rate --
   both measured net losses on HW).  Each tile's edges are sorted by src so
   the gather walks ascending HBM addresses.
 - one-hot scatter/select matrices are baked on the host in fp8 and stay
   resident in SBUF (both orientations), eliminating the per-chunk DVE
   is_equal builds.  Mixed-dtype matmul (fp8 stationary x bf16 moving) does
   the scatter-add / attention-select work.
 - the per-edge softmax-weight multiply runs as one batched DVE op per ~5
   chunks (2x mode, amortized access latency) instead of per-chunk ops.
 - per-node a_s/a_d come from two extra small matmuls in the transform
   (x @ Wsd with Wsd = contract(W, att) built on device), not DVE
   mult+reduce; they ride the gather row / stay resident in SBUF.
 - a_e (edge-attr attention term) for both layers is precomputed into SBUF
   during the startup window and added to the logits with one eye-matmul.
 - layer-1 node transform is fused into the layer-0 message consumer
   (transpose + matmul per tile), removing a serial phase and a DRAM
   round-trip of the intermediate features.
 - partial logits are computed per-core before the final AllReduce, which
   then moves [G, CLS] instead of [G, H*C].

kernel(**inputs) accepts the full (unsharded) inputs of reference.setup_inputs()
and returns the full [G, CLS] log-softmax output.
"""
import sys
sys.path.insert(0, "/opt/trn_rl_repo")

import numpy as np
import ml_dtypes

import concourse.bacc as bacc
import concourse.bass as bass
import concourse.mybir as mybir
import concourse.tile as tile
from concourse import library_config
from concourse.bass_utils import run_bass_kernel_spmd

F32 = mybir.dt.float32
BF16 = mybir.dt.bfloat16
FP8 = mybir.dt.float8e4
U8 = mybir.dt.uint8
I16 = mybir.dt.int16
NEG_ATT = 0.2
NEG_ACT = 0.01
AF = mybir.ActivationFunctionType
OP = mybir.AluOpType


def cdiv(a, b):
    return (a + b - 1) // b


class Cfg:
    def __init__(self, N=20000, E=320000, F_IN=256, ED=16, H=8, C=64, G=64,
                 CLS=10, ncores=8, h8=False):
        self.N, self.E, self.F_IN, self.ED = N, E, F_IN, ED
        self.H, self.C, self.G, self.CLS = H, C, G, CLS
        self.ncores = ncores
        self.HC = H * C
        self.NPC = cdiv(N, ncores)            # real nodes per core
        self.NT = cdiv(self.NPC, 128)         # 128-node dst tiles per core
        self.NPCP = self.NT * 128             # padded nodes per core
        self.NP = ncores * self.NPCP          # padded global nodes
        assert self.NP < 32768, "gather indices must fit int16"
        assert F_IN % 128 == 0 and self.HC % 128 == 0
        self.KX = F_IN // 128
        self.KH = self.HC // 128
        # gather row, always a bf16-typed table (8-bit gathers run at half
        # descriptor rate).  h8=False: [h bf16 (HC) | a_s (H) | a_d (H) |
        # pad].  h8=True: h is stored as fp8e4 bytes bitcast into the first
        # HC/2 bf16 elements (40% less gather traffic, ~1.6e-4 extra err,
        # but the DVE weight-multiply drops from 2x to 1x mode).
        self.h8 = h8
        self.HE = self.HC // 2 if h8 else self.HC   # h element count in row
        self.HROW = cdiv(self.HE + 2 * self.H, 128) * 128
        # build the U/D one-hot (ohA) on DVE per chunk instead of keeping it
        # resident in SBUF (frees 42.5 KB/partition for a 3rd gather buffer,
        # but costs ~3 us/tile of DVE; measured neutral-to-worse)
        self.ohadve = False
        # c-major feature permutation: new col f = c*H+h holds old col h*C+c
        self.cm = (np.arange(self.HC) % H) * C + (np.arange(self.HC) // H)


def _wrap16(arr_i16, nch_list):
    """Per-tile wrap of an int16 index list into the [128, n/16] SWDGE layout."""
    cols = []
    off = 0
    for nch in nch_list:
        n = nch * 128
        seg = arr_i16[off:off + n]
        cols.append(np.tile(seg.reshape(n // 16, 16).T, (8, 1)))
        off += n
    return np.ascontiguousarray(np.concatenate(cols, axis=1))


def preprocess(cfg, inputs):
    """Host-side sharding: sort edges by dst, bucket into per-core dst tiles,
    build index / one-hot helper tables.  All floating-point math on tensor
    data stays on device (host only permutes / builds 0-1 indicator tables)."""
    c = cfg
    x = np.asarray(inputs["x"], np.float32)
    ei = np.asarray(inputs["edge_index"]).astype(np.int64)
    ea = np.asarray(inputs["edge_attr"], np.float32)
    batch = np.asarray(inputs["batch"]).astype(np.int64)
    src, dst = ei[0], ei[1]

    order = np.argsort(dst, kind="stable")
    ds, ss, eap = dst[order], src[order], ea[order]
    core = ds // c.NPC
    local = ds - core * c.NPC
    tilei = local // 128
    dstl = local % 128

    cnt = np.zeros((c.ncores, c.NT), np.int64)
    np.add.at(cnt, (core, tilei), 1)
    nch = np.maximum(1, (cnt.max(axis=0) + 127) // 128)   # chunks per tile slot
    NCH = int(nch.sum())
    TOT = NCH * 128
    slot0 = np.concatenate([[0], np.cumsum(nch)])

    starts = np.concatenate([[0], np.cumsum(cnt.reshape(-1))])

    def gid(n):
        cc = n // c.NPC
        return cc * c.NPCP + (n - cc * c.NPC)

    per_core = []
    for cc in range(c.ncores):
        srcidx = np.zeros(TOT, np.int16)
        dstlv = np.full(TOT, 200, np.int64)    # padding edges: no one-hot row
        ea_rows = np.zeros((TOT, c.ED), np.float32)
        for t in range(c.NT):
            k = cc * c.NT + t
            s, e = starts[k], starts[k + 1]
            n = e - s
            o = slot0[t] * 128
            if n:
                # sort the tile's edges by src so the gather walks ascending
                # addresses (HBM row locality); all per-edge tables follow
                # the same permutation, so any order is valid.
                so = np.argsort(ss[s:e], kind="stable")
                srcidx[o:o + n] = gid(ss[s:e][so]).astype(np.int16)
                dstlv[o:o + n] = dstl[s:e][so]
                ea_rows[o:o + n] = eap[s:e][so]
        meta = _wrap16(srcidx, nch)                          # [128, NCH*8] i16
        # one-hot tables, fp8 (exactly representable 0/1), both orientations:
        # per chunk 256 cols: [ohA (e->dst) | ohB = ohA^T (dst->e)]
        ohtab = np.zeros((128, NCH * 256), ml_dtypes.float8_e4m3)
        ohtabB = np.zeros((128, NCH * 128), ml_dtypes.float8_e4m3)
        for k in range(NCH):
            dv = dstlv[k * 128:(k + 1) * 128]
            valid = dv < 128
            e_idx = np.nonzero(valid)[0]
            d_idx = dv[valid]
            ohtab[e_idx, k * 256 + d_idx] = 1.0              # ohA[e, d]
            ohtab[d_idx, k * 256 + 128 + e_idx] = 1.0        # ohB[d, e]
            ohtabB[d_idx, k * 128 + e_idx] = 1.0
        # per-edge dst-local values for the DVE-built ohA (padding edges get
        # 200.0 which never matches iota 0..127)
        dstlf = dstlv.astype(np.float32).reshape(NCH, 128).T  # [128, NCH]
        eaT = np.ascontiguousarray(ea_rows.T)                # [ED, TOT]

        nreal = min(c.NPC, c.N - cc * c.NPC)
        xt = np.zeros((c.F_IN, c.NPCP), np.float32)
        xt[:, :nreal] = x[cc * c.NPC: cc * c.NPC + nreal].T
        goneh = np.zeros((c.NPCP, c.G), np.float32)
        bslice = batch[cc * c.NPC: cc * c.NPC + nreal]
        goneh[np.arange(nreal), bslice] = 1.0

        per_core.append(dict(
            xt=xt.astype(ml_dtypes.bfloat16),
            meta=meta,
            ohtab=ohtab.view(np.uint8),
            ohtabB=ohtabB.view(np.uint8),
            dstlf=np.ascontiguousarray(dstlf),
            eat=eaT.astype(ml_dtypes.bfloat16),
            goneh=goneh.astype(ml_dtypes.bfloat16),
        ))

    gcnt = np.bincount(batch, minlength=c.G).astype(np.float32)
    invcnt = (1.0 / np.maximum(gcnt, 1.0)).reshape(c.G, 1).astype(np.float32)

    def rep(v, rows, perm=None):
        v = np.asarray(v, np.float32).reshape(1, -1)
        if perm is not None:
            v = v[:, perm]
        return np.tile(v, (rows, 1))

    cm = c.cm
    w0 = np.asarray(inputs["W0"], np.float32)[:, cm]
    w1 = np.asarray(inputs["W1"], np.float32)[cm][:, cm]
    wlin = np.asarray(inputs["Wlin"], np.float32)[cm, :]
    bf = ml_dtypes.bfloat16

    shared = dict(
        w0=w0.astype(bf),
        w1=w1.astype(bf),
        wlin=wlin.astype(bf),
        blin=np.asarray(inputs["blin"], np.float32).reshape(1, c.CLS).astype(bf),
        b0r=rep(inputs["b0"], 128, cm).astype(bf),
        b1r=rep(inputs["b1"], 128, cm).astype(bf),
        atts0=rep(inputs["att_src0"], 128, cm).astype(bf),
        atd0=rep(inputs["att_dst0"], 128, cm).astype(bf),
        atts1=rep(inputs["att_src1"], 128, cm).astype(bf),
        atd1=rep(inputs["att_dst1"], 128, cm).astype(bf),
        ate0=rep(inputs["att_edge0"], c.ED),
        ate1=rep(inputs["att_edge1"], c.ED),
        we0=np.asarray(inputs["We0"], np.float32),
        we1=np.asarray(inputs["We1"], np.float32),
        eye=np.eye(128, dtype=np.float32).astype(bf),
        iota=np.tile(np.arange(128, dtype=np.float32), (128, 1)).astype(bf),
        ones1=np.ones((1, 128), np.float32).astype(bf),
        invcnt=invcnt,
        bling=np.tile(np.asarray(inputs["blin"], np.float32).reshape(1, c.CLS),
                      (c.G, 1)),
    )
    in_maps = [{**pc, **shared} for pc in per_core]
    return in_maps, [int(v) for v in nch]


def build(cfg, nch, collectives=True, repeat=1):
    c = cfg
    H = c.H
    NCH = sum(nch)
    TOT = NCH * 128
    slot0 = np.concatenate([[0], np.cumsum(nch)]).astype(int)
    rg = [list(range(c.ncores))]

    nc = bacc.Bacc("TRN2", target_bir_lowering=False, debug=False,
                   num_devices=c.ncores)

    def EI(name, shape, dt):
        return nc.dram_tensor(name, list(shape), dt, kind="ExternalInput")

    xt_e = EI("xt", (c.F_IN, c.NPCP), BF16)
    meta_e = EI("meta", (128, NCH * 8), I16)
    if c.ohadve:
        ohtab_e = EI("ohtabB", (128, NCH * 128), U8)
        dstlf_e = EI("dstlf", (128, NCH), F32)
    else:
        ohtab_e = EI("ohtab", (128, NCH * 256), U8)
    eat_e = EI("eat", (c.ED, TOT), BF16)
    goneh_e = EI("goneh", (c.NPCP, c.G), BF16)
    w0_e = EI("w0", (c.F_IN, c.HC), BF16)
    w1_e = EI("w1", (c.HC, c.HC), BF16)
    wlin_e = EI("wlin", (c.HC, c.CLS), BF16)
    blin_e = EI("blin", (1, c.CLS), BF16)
    b0r_e = EI("b0r", (128, c.HC), BF16)
    b1r_e = EI("b1r", (128, c.HC), BF16)
    atts0_e = EI("atts0", (128, c.HC), BF16)
    atd0_e = EI("atd0", (128, c.HC), BF16)
    atts1_e = EI("atts1", (128, c.HC), BF16)
    atd1_e = EI("atd1", (128, c.HC), BF16)
    ate0_e = EI("ate0", (c.ED, c.HC), F32)
    ate1_e = EI("ate1", (c.ED, c.HC), F32)
    we0_e = EI("we0", (c.ED, c.HC), F32)
    we1_e = EI("we1", (c.ED, c.HC), F32)
    eye_e = EI("eye", (128, 128), BF16)
    iota_e = EI("iota", (128, 128), BF16)
    ones1_e = EI("ones1", (1, 128), BF16)
    invcnt_e = EI("invcnt", (c.G, 1), F32)
    bling_e = EI("bling", (c.G, c.CLS), F32)

    out_e = nc.dram_tensor("out", [c.G, c.CLS], F32, kind="ExternalOutput")

    def cmaj(ap):
        """view a [128, HC] c-major AP as [128, C, H] (innermost unit-stride)"""
        return ap.rearrange("p (cc h) -> p cc h", h=H)

    def hview(ap):
        """view a [128, HC] c-major AP as [128, H, C] (strided head-major)"""
        return ap.rearrange("p (cc h) -> p h cc", h=H)

    with tile.TileContext(nc, num_cores=c.ncores) as tc:
        import contextlib
        with contextlib.ExitStack() as stack:
            cpool = stack.enter_context(tc.tile_pool(name="consts", bufs=1))
            dram = stack.enter_context(tc.tile_pool(name="dram", bufs=1, space="DRAM"))
            ppool = stack.enter_context(tc.tile_pool(name="ppersist", bufs=1, space="PSUM"))

            nc.gpsimd.load_library(library_config.mlp)

            def load_const(ext, shape, dt, name, engine=None):
                tl = cpool.tile(list(shape), dt, tag=name)
                (engine or nc.sync).dma_start(tl[:], ext[:])
                return tl

            eye = load_const(eye_e, (128, 128), BF16, "eye")
            ones1 = load_const(ones1_e, (1, 128), BF16, "ones1")
            blin = load_const(blin_e, (1, c.CLS), BF16, "blin")
            b0r = load_const(b0r_e, (128, c.HC), BF16, "b0r")
            b1r = load_const(b1r_e, (128, c.HC), BF16, "b1r")
            atts0 = load_const(atts0_e, (128, c.HC), BF16, "atts0")
            atd0 = load_const(atd0_e, (128, c.HC), BF16, "atd0")
            atts1 = load_const(atts1_e, (128, c.HC), BF16, "atts1")
            atd1 = load_const(atd1_e, (128, c.HC), BF16, "atd1")
            invcnt = load_const(invcnt_e, (c.G, 1), F32, "invcnt")
            bling = load_const(bling_e, (c.G, c.CLS), F32, "bling")
            goneh_sb = cpool.tile([128, c.NT, c.G], BF16, tag="goneh")
            nc.scalar.dma_start(
                goneh_sb[:], goneh_e.ap().rearrange("(t p) g -> p t g", p=128))
            meta = load_const(meta_e, (128, NCH * 8), I16, "meta",
                              engine=nc.gpsimd)
            ohcols = NCH * 128 if c.ohadve else NCH * 256
            ohtab = load_const(ohtab_e, (128, ohcols), U8, "ohtab",
                               engine=nc.gpsimd)
            if c.ohadve:
                dstlf = load_const(dstlf_e, (128, NCH), F32, "dstlf",
                                   engine=nc.gpsimd)
                iota = load_const(iota_e, (128, 128), BF16, "iota")

            def load_chunks(ext, kparts, cols, name):
                tiles = []
                for k in range(kparts):
                    tl = cpool.tile([128, cols], BF16, tag=f"{name}{k}")
                    nc.sync.dma_start(tl[:], ext[k * 128:(k + 1) * 128, :])
                    tiles.append(tl)
                return tiles

            w0c = load_chunks(w0_e, c.KX, c.HC, "w0")
            w1c = load_chunks(w1_e, c.KH, c.HC, "w1")
            wlc = load_chunks(wlin_e, c.KH, c.CLS, "wl")

            # small staging pool for eat slices; stays open all program so the
            # message pools never reuse (and WAR-serialize on) its space
            aesb = stack.enter_context(tc.tile_pool(name="aestage", bufs=1))

            # startup-only scratch pool for the ve / wsd weight contractions
            prep_pool = tc.tile_pool(name="prep", bufs=1)
            prep = prep_pool.__enter__()

            # Ve[l] = contract(We[l], att_edge[l]) over C  -> [ED, H]
            def make_ve(we_ext, ate_ext, name):
                wet = prep.tile([c.ED, c.H, c.C], F32, tag="prepw")
                nc.sync.dma_start(wet[:], we_ext.ap().rearrange("d (h cc) -> d h cc", h=H))
                atet = prep.tile([c.ED, c.H, c.C], F32, tag="prepa")
                nc.sync.dma_start(atet[:], ate_ext.ap().rearrange("d (h cc) -> d h cc", h=H))
                prod = prep.tile([c.ED, c.H, c.C], F32, tag="prepp")
                nc.vector.tensor_tensor(prod[:], wet[:], atet[:], OP.mult)
                ve32 = prep.tile([c.ED, c.H], F32, tag="prep3")
                nc.vector.tensor_reduce(ve32[:], prod[:], axis=mybir.AxisListType.X,
                                        op=OP.add)
                vef = cpool.tile([c.ED, c.H], BF16, tag=name)
                nc.vector.tensor_copy(vef[:], ve32[:])
                return vef

            ve0 = make_ve(we0_e, ate0_e, "ve0")
            ve1 = make_ve(we1_e, ate1_e, "ve1")

            # Wsd[l][k] = [128, 2H]: cols 0:H = contract(w[k], atts) over C,
            # cols H:2H = contract(w[k], atd).  a_s/a_d then come from matmuls.
            wsd_tmp = prep.tile([128, c.HC], F32, tag="wsdtmp")
            wsd_r32 = prep.tile([128, 2 * c.H], F32, tag="wsdr32")

            def make_wsd(w_tiles, atts, atd, name):
                outs = []
                for k, wt in enumerate(w_tiles):
                    sd = cpool.tile([128, 2 * c.H], BF16, tag=f"{name}{k}")
                    nc.vector.tensor_tensor(wsd_tmp[:], wt[:], atts[:], OP.mult)
                    nc.vector.tensor_reduce(wsd_r32[:, 0:c.H], hview(wsd_tmp[:]),
                                            axis=mybir.AxisListType.X, op=OP.add)
                    nc.vector.tensor_tensor(wsd_tmp[:], wt[:], atd[:], OP.mult)
                    nc.vector.tensor_reduce(wsd_r32[:, c.H:2 * c.H],
                                            hview(wsd_tmp[:]),
                                            axis=mybir.AxisListType.X, op=OP.add)
                    nc.vector.tensor_copy(sd[:], wsd_r32[:])
                    outs.append(sd)
                return outs

            wsd0 = make_wsd(w0c, atts0, atd0, "wsd0")
            wsd1 = make_wsd(w1c, atts1, atd1, "wsd1")
            prep_pool.__exit__(None, None, None)

            # a_s/a_d per (tile, node): [128, NT, 2H] bf16, one buffer per layer
            asad0_sb = cpool.tile([128, c.NT, 2 * c.H], BF16, tag="asad0")
            asad1_sb = cpool.tile([128, c.NT, 2 * c.H], BF16, tag="asad1")
            # a_e per (edge, head): [128, NCH*H] bf16, one per layer
            ae0_sb = cpool.tile([128, NCH * c.H], BF16, tag="ae0")
            ae1_sb = cpool.tile([128, NCH * c.H], BF16, tag="ae1")
            ae_sb = [ae0_sb, ae1_sb]

            h0_loc = dram.tile([c.NPCP, c.HROW], BF16)
            h1_loc = dram.tile([c.NPCP, c.HROW], BF16)
            pool_in = dram.tile([c.G, c.CLS], F32)
            pool_out = dram.tile([c.G, c.CLS], F32, addr_space="Shared")

            poolP = ppool.tile([c.G, c.HC], F32, tag="poolP")
            aeP = ppool.tile([128, max(nch) * H], F32, tag="aeP")

            # ---- startup: precompute a_e for both layers -------------------
            # (issued after transform0 so the ACT/PE queues drain the
            # transform first; overlaps the AllGather0 window)
            def ae_prep():
                step = cdiv(c.NT, 8)
                maxcols = step * max(nch) * 128
                bounds = [(i, min(c.NT, i + step))
                          for i in range(0, c.NT, step)]
                for t0, t1 in bounds:
                    cols = (slot0[t1] - slot0[t0]) * 128
                    eat_sb = aesb.tile([c.ED, maxcols], BF16, tag="eat")
                    nc.sync.dma_start(
                        eat_sb[:, 0:cols],
                        eat_e[:, slot0[t0] * 128:slot0[t1] * 128])
                    for l, ve in enumerate((ve0, ve1)):
                        for t in range(t0, t1):
                            n = nch[t]
                            for ch in range(n):
                                col = (slot0[t] - slot0[t0] + ch) * 128
                                nc.tensor.matmul(
                                    aeP[:, ch * H:(ch + 1) * H],
                                    eat_sb[:, col:col + 128], ve[:],
                                    start=True, stop=True)
                            nc.scalar.copy(
                                ae_sb[l][:, slot0[t] * H:slot0[t + 1] * H],
                                aeP[:, 0:n * H])

            # ---- phase: layer-0 node transform -----------------------------
            def transform0():
                with tc.tile_pool(name="tf", bufs=3) as sb, \
                     tc.tile_pool(name="tfp", bufs=2, space="PSUM") as pp:
                    for t in range(c.NT):
                        hp = pp.tile([128, c.HC], F32, tag="hp")
                        sdp = pp.tile([128, 2 * c.H], F32, tag="sdp")
                        lh = sb.tile([128, c.KX, 128], BF16, tag="lh")
                        nc.sync.dma_start(
                            lh[:], xt_e[:, t * 128:(t + 1) * 128]
                            .rearrange("(k p) j -> p k j", p=128))
                        for k in range(c.KX):
                            nc.tensor.matmul(hp[:], lh[:, k, :], w0c[k][:],
                                             start=(k == 0), stop=(k == c.KX - 1))
                            nc.tensor.matmul(sdp[:], lh[:, k, :], wsd0[k][:],
                                             start=(k == 0), stop=(k == c.KX - 1))
                        hrow = sb.tile([128, c.HROW], BF16, tag="hrow")
                        if c.h8:
                            nc.scalar.copy(hrow[:, 0:c.HE].bitcast(FP8), hp[:])
                        else:
                            nc.scalar.copy(hrow[:, 0:c.HE], hp[:])
                        nc.scalar.copy(hrow[:, c.HE:c.HE + 2 * c.H], sdp[:])
                        if c.HROW > c.HE + 2 * c.H:
                            nc.vector.memset(hrow[:, c.HE + 2 * c.H:], 0.0)
                        nc.scalar.copy(asad0_sb[:, t, :], sdp[:])
                        nc.sync.dma_start(
                            h0_loc[t * 128:(t + 1) * 128, :], hrow[:])

            # ---- phase: edge message passing -------------------------------
            NMAX = max(nch)

            def message(h_full, ae, asad_sb, ve, brep, consumer, pools):
                gb, sb, eb, ob, pU, pD, pQ = pools
                for t in range(c.NT):
                    n = nch[t]
                    gh = gb.tile([128, n, c.HROW], BF16, tag="gh")
                    nc.gpsimd.dma_gather(
                        gh[:], h_full[:],
                        meta[:, slot0[t] * 8:(slot0[t] + n) * 8],
                        n * 128, n * 128, c.HROW, single_packet=False)
                    ghS = gh[:, :, c.HE:c.HE + c.H]
                    adt = asad_sb[:, t, c.H:2 * c.H]
                    # attention logits for the whole tile: q = a_s+a_e+a_d
                    qa = pQ.tile([128, n * H], F32, tag="qa")
                    nc.tensor.matmul(qa[:], eye[:], ghS,
                                     start=True, stop=False,
                                     skip_group_check=True)
                    nc.tensor.matmul(qa[:], eye[:],
                                     ae[:, slot0[t] * H:slot0[t + 1] * H],
                                     start=False, stop=False,
                                     skip_group_check=True)
                    for ch in range(n):
                        if c.ohadve:
                            ohB = ohtab[:, (slot0[t] + ch) * 128:
                                        (slot0[t] + ch) * 128 + 128].bitcast(FP8)
                        else:
                            ohB = ohtab[:, (slot0[t] + ch) * 256 + 128:
                                        (slot0[t] + ch) * 256 + 256].bitcast(FP8)
                        nc.tensor.matmul(qa[:, ch * H:(ch + 1) * H], ohB,
                                         adt, start=False, stop=(ch == n - 1),
                                         skip_group_check=True)
                    # exp(leaky_relu(q)) = max(exp(q), exp(0.2 q)), batched
                    e1 = sb.tile([128, n * H], BF16, tag="e1")
                    e2 = sb.tile([128, n * H], BF16, tag="e2")
                    exa = sb.tile([128, n * H], BF16, tag="exa")
                    nc.scalar.activation(e1[:], qa[:], AF.Exp)
                    nc.scalar.activation(e2[:], qa[:], AF.Exp, scale=NEG_ATT)
                    nc.vector.tensor_max(exa[:], e1[:], e2[:])
                    U = pU.tile([128, c.HC], F32, tag="U")
                    D = pD.tile([128, c.H], F32, tag="D")
                    # batched per-edge weight multiply: one DVE op per group
                    # of chunks (amortizes the per-op SBUF access latency)
                    GRP = 5
                    for g0 in range(0, n, GRP):
                        g1 = min(n, g0 + GRP)
                        exh_g = eb.tile([128, GRP, c.C, c.H], BF16, tag="exhg")
                        ghHg = gh[:, g0:g1, 0:c.HE]
                        if c.h8:
                            ghHg = ghHg.bitcast(FP8)
                        nc.vector.tensor_tensor(
                            exh_g[:, 0:g1 - g0],
                            ghHg.rearrange("p n (cc h) -> p n cc h", h=H),
                            exa[:, g0 * H:g1 * H]
                            .rearrange("p (n h) -> p n h", h=H)
                            .unsqueeze(2)
                            .broadcast_to([128, g1 - g0, c.C, c.H]),
                            OP.mult)
                        for ch in range(g0, g1):
                            if c.ohadve:
                                oha = ob.tile([128, 128], BF16, tag="oha")
                                nc.vector.tensor_scalar(
                                    oha[:], iota[:],
                                    dstlf[:, slot0[t] + ch:slot0[t] + ch + 1],
                                    None, OP.is_equal)
                                ohA = oha[:]
                            else:
                                ohA = ohtab[:, (slot0[t] + ch) * 256:
                                            (slot0[t] + ch) * 256 + 128
                                            ].bitcast(FP8)
                            exs = exa[:, ch * H:(ch + 1) * H]
                            nc.tensor.matmul(U[:], ohA, exh_g[:, ch - g0],
                                             start=(ch == 0), stop=(ch == n - 1))
                            nc.tensor.matmul(D[:], ohA, exs,
                                             start=(ch == 0), stop=(ch == n - 1))
                    # tile epilogue: out = U / (D + eps) + b
                    rdt = sb.tile([128, c.H], F32, tag="rdt")
                    nc.vector.tensor_single_scalar(rdt[:], D[:], 1e-16, OP.add)
                    rd32 = sb.tile([128, c.H], F32, tag="rd32")
                    nc.vector.reciprocal(rd32[:], rdt[:])
                    rd = sb.tile([128, c.H], BF16, tag="rd")
                    nc.vector.tensor_copy(rd[:], rd32[:])
                    ub = sb.tile([128, c.HC], BF16, tag="ub")
                    nc.scalar.copy(ub[:], U[:])
                    o1 = sb.tile([128, c.C, c.H], BF16, tag="o1")
                    nc.vector.tensor_tensor(
                        o1[:], cmaj(ub[:]),
                        rd[:].unsqueeze(1).broadcast_to([128, c.C, c.H]),
                        OP.mult)
                    o2 = sb.tile([128, c.HC], BF16, tag="o2")
                    nc.vector.tensor_tensor(
                        o2[:], o1[:].rearrange("p cc h -> p (cc h)"),
                        brep[:], OP.add)
                    consumer(t, o2, sb)

            # consume0: leaky-relu + fused layer-1 transform for this tile
            def make_consume0(pT, pp1):
                def consume0(t, o2, sb):
                    f1 = sb.tile([128, c.HC], BF16, tag="f1")
                    nc.vector.scalar_tensor_tensor(f1[:], o2[:], NEG_ACT, o2[:],
                                                   OP.mult, OP.max)
                    tp = pT.tile([128, c.KH, 128], BF16, tag="tp")
                    tpc = sb.tile([128, c.KH, 128], BF16, tag="tpc")
                    for k in range(c.KH):
                        nc.tensor.transpose(tp[:, k, :],
                                            f1[:, k * 128:(k + 1) * 128], eye[:])
                        nc.scalar.copy(tpc[:, k, :], tp[:, k, :])
                    h1p = pp1.tile([128, c.HC], F32, tag="h1p")
                    sd1p = pp1.tile([128, 2 * c.H], F32, tag="sd1p")
                    for k in range(c.KH):
                        nc.tensor.matmul(h1p[:], tpc[:, k, :], w1c[k][:],
                                         start=(k == 0), stop=(k == c.KH - 1))
                        nc.tensor.matmul(sd1p[:], tpc[:, k, :], wsd1[k][:],
                                         start=(k == 0), stop=(k == c.KH - 1))
                    hrow = sb.tile([128, c.HROW], BF16, tag="h1row")
                    if c.h8:
                        nc.scalar.copy(hrow[:, 0:c.HE].bitcast(FP8), h1p[:])
                    else:
                        nc.scalar.copy(hrow[:, 0:c.HE], h1p[:])
                    nc.scalar.copy(hrow[:, c.HE:c.HE + 2 * c.H], sd1p[:])
                    if c.HROW > c.HE + 2 * c.H:
                        nc.vector.memset(hrow[:, c.HE + 2 * c.H:], 0.0)
                    nc.scalar.copy(asad1_sb[:, t, :], sd1p[:])
                    nc.sync.dma_start(
                        h1_loc[t * 128:(t + 1) * 128, :], hrow[:])
                return consume0

            def consume1(t, o2, sb):
                h2 = sb.tile([128, c.HC], BF16, tag="h2")
                nc.vector.scalar_tensor_tensor(h2[:], o2[:], NEG_ACT, o2[:],
                                               OP.mult, OP.max)
                nc.tensor.matmul(poolP[:], goneh_sb[:, t, :], h2[:],
                                 start=(t == 0), stop=(t == c.NT - 1))

            def allgather(loc, full):
                if collectives:
                    nc.gpsimd.collective_compute("AllGather", OP.bypass,
                                                 ins=[loc.opt()], outs=[full.opt()],
                                                 replica_groups=rg)
                else:  # single-core profiling stand-in
                    nc.gpsimd.dma_start(full[0:c.NPCP, :], loc[:])

            for _rep in range(repeat):
                h0_full = dram.tile([c.NP, c.HROW], BF16, addr_space="Shared",
                                    tag=f"h0f{_rep}")
                h1_full = dram.tile([c.NP, c.HROW], BF16, addr_space="Shared",
                                    tag=f"h1f{_rep}")
                transform0()
                if _rep == 0:
                    ae_prep()
                allgather(h0_loc, h0_full)
                GB = 3 if c.ohadve else 2
                with tc.tile_pool(name="mg", bufs=GB) as gb, \
                     tc.tile_pool(name="ms", bufs=3) as sb, \
                     tc.tile_pool(name="me", bufs=3) as eb, \
                     tc.tile_pool(name="mo", bufs=4) as ob, \
                     tc.tile_pool(name="mu", bufs=1, space="PSUM") as pU, \
                     tc.tile_pool(name="md", bufs=1, space="PSUM") as pD, \
                     tc.tile_pool(name="mq", bufs=1, space="PSUM") as pQ, \
                     tc.tile_pool(name="mt", bufs=1, space="PSUM") as pT, \
                     tc.tile_pool(name="mh1", bufs=1, space="PSUM") as pp1:
                    message(h0_full, ae_sb[0], asad0_sb, ve0, b0r,
                            make_consume0(pT, pp1),
                            (gb, sb, eb, ob, pU, pD, pQ))
                allgather(h1_loc, h1_full)
                with tc.tile_pool(name="ng", bufs=GB) as gb, \
                     tc.tile_pool(name="ns", bufs=3) as sb, \
                     tc.tile_pool(name="ne", bufs=3) as eb, \
                     tc.tile_pool(name="no", bufs=4) as ob, \
                     tc.tile_pool(name="nu", bufs=2, space="PSUM") as pU, \
                     tc.tile_pool(name="nd", bufs=1, space="PSUM") as pD, \
                     tc.tile_pool(name="nq", bufs=2, space="PSUM") as pQ:
                    message(h1_full, ae_sb[1], asad1_sb, ve1, b1r,
                            consume1, (gb, sb, eb, ob, pU, pD, pQ))

            # ---- pooling + classifier + log_softmax ------------------------
            # partial logits are computed per-core BEFORE the AllReduce
            # (linear), so the collective moves [G, CLS] instead of [G, HC]
            # and the classifier matmuls leave the post-collective path.
            with tc.tile_pool(name="fin", bufs=2) as sb, \
                 tc.tile_pool(name="finp", bufs=2, space="PSUM") as pp:
                pooled = sb.tile([c.G, c.HC], BF16, tag="pooled")
                nc.scalar.activation(pooled[:], poolP[:], AF.Copy,
                                     scale=invcnt[:])
                lg = pp.tile([c.G, c.CLS], F32, tag="lg")
                for k in range(c.KH):
                    tp = pp.tile([128, c.G], BF16, tag="ftp")
                    nc.tensor.transpose(tp[:], pooled[:, k * 128:(k + 1) * 128],
                                        eye[0:c.G, 0:c.G])
                    tpc = sb.tile([128, c.G], BF16, tag="ftpc")
                    nc.scalar.copy(tpc[:], tp[:])
                    nc.tensor.matmul(lg[:], tpc[:], wlc[k][:],
                                     start=(k == 0), stop=(k == c.KH - 1))
                lgp = sb.tile([c.G, c.CLS], F32, tag="lgp")
                nc.scalar.copy(lgp[:], lg[:])
                nc.sync.dma_start(pool_in[:], lgp[:])
                if collectives:
                    nc.gpsimd.collective_compute("AllReduce", OP.add,
                                                 ins=[pool_in.opt()],
                                                 outs=[pool_out.opt()],
                                                 replica_groups=rg)
                else:
                    nc.sync.dma_start(pool_out[:], pool_in[:])
                lgr = sb.tile([c.G, c.CLS], F32, tag="lgr")
                nc.sync.dma_start(lgr[:], pool_out[:])
                lgs = sb.tile([c.G, c.CLS], F32, tag="lgs")
                nc.vector.tensor_tensor(lgs[:], lgr[:], bling[:], OP.add)
                mx = sb.tile([c.G, 1], F32, tag="mx")
                nc.vector.tensor_reduce(mx[:], lgs[:], axis=mybir.AxisListType.X,
                                        op=OP.max)
                zc = sb.tile([c.G, c.CLS], F32, tag="zc")
                nc.vector.tensor_scalar(zc[:], lgs[:], mx[:], None, OP.subtract)
                ez = sb.tile([c.G, c.CLS], F32, tag="ez")
                se = sb.tile([c.G, 1], F32, tag="se")
                nc.scalar.activation(ez[:], zc[:], AF.Exp, accum_out=se[:])
                lse = sb.tile([c.G, 1], F32, tag="lse")
                nc.scalar.activation(lse[:], se[:], AF.Ln)
                osb = sb.tile([c.G, c.CLS], F32, tag="osb")
                nc.vector.tensor_scalar(osb[:], zc[:], lse[:], None, OP.subtract)
                nc.sync.dma_start(out_e[:], osb[:])

    nc.compile()
    return nc


_CACHE = {}


def _get_program(cfg, nch):
    key = (cfg.N, cfg.E, cfg.F_IN, cfg.ED, cfg.H, cfg.C, cfg.G, cfg.CLS,
           cfg.ncores, cfg.h8, tuple(nch))
    if key not in _CACHE:
        _CACHE[key] = build(cfg, nch)
    return _CACHE[key]


def run(inputs, cfg=None):
    cfg = cfg or Cfg()
    in_maps, nch = preprocess(cfg, inputs)
    nc = _get_program(cfg, nch)
    res = run_bass_kernel_spmd(nc, in_maps, list(range(cfg.ncores)))
    return res.results[0]["out"].astype(np.float32)


def kernel(**inputs) -> np.ndarray:
    return run(inputs)
